# revision 4
# baseline (speedup 1.0000x reference)
"""Trainium2 Bass kernel for nn_AutoReconstruction.

Computes out[b, m] = dot(inputs[b, m, :], W[m, :]) + bias[m]
  inputs: [1024, 2048, 128] f32, W: [2048, 128] f32, bias: [2048] f32
  out:    [1024, 2048] f32

Sharding: batch dim B=1024 split across 8 NeuronCores (128 each); W/bias
replicated. Per-core traffic ~129 MiB -> memory-bound (~360 GB/s/core).

Per-core algorithm (M split into C=16 chunks of 128; m = c*128 + p):
  - X[b] loaded as SBUF tile [p=128, c, i] (contiguous 512B runs, 2 MiB DMAs)
  - 1/3 of batches: DVE tensor_tensor_reduce fuses mult+reduce+bias-init
  - 2/3 of batches: DVE tensor_mul -> ScalarE activation(Identity,
    bias=bias/128, accum_out) so both engines run at ~the DMA roofline
  - results accumulate in SBUF as acc[m_p, b, c]; 16 PE transposes flip to
    [b_p, m] for one contiguous 1 MiB output DMA
"""

import numpy as np

B, M, I = 1024, 2048, 128
NCORES = 8
BLOC = B // NCORES  # 128 batches per core
C = M // 128        # 16 m-chunks
NB = 2              # batches per input DMA (2 MiB transfers)

_CACHE = {}
LAST_RESULT = None

_AXON_PJRT_SO = "/opt/axon/libaxon_pjrt.so"


def _ensure_ntff_hook():
    """Provide antenv.axon_hooks if the image lacks it.

    concourse.bass_utils unconditionally imports
    antenv.axon_hooks.get_axon_ntff_profile_hook when trace=True under
    axon; some images ship antenv without that submodule. Register a
    synthetic module wired to libaxon_pjrt.so's NRT-profile C ABI (or a
    None hook, which bass_utils degrades on gracefully).
    """
    import sys
    try:
        from antenv.axon_hooks import get_axon_ntff_profile_hook  # noqa: F401
        return
    except ImportError:
        pass
    import contextlib
    import ctypes
    import types

    hook = None
    try:
        lib = ctypes.CDLL(_AXON_PJRT_SO)
        if hasattr(lib, "axon_start_nrt_profile"):
            lib.axon_start_nrt_profile.argtypes = [
                ctypes.POINTER(ctypes.c_int64), ctypes.c_size_t]
            lib.axon_start_nrt_profile.restype = ctypes.c_int64
            lib.axon_stop_nrt_profile.argtypes = [ctypes.c_char_p]
            lib.axon_stop_nrt_profile.restype = ctypes.c_int64

            @contextlib.contextmanager
            def _hook(output_dir, device_ids):
                import jax
                jax.devices()
                if device_ids:
                    ids = (ctypes.c_int64 * len(device_ids))(*device_ids)
                    rc = lib.axon_start_nrt_profile(ids, len(device_ids))
                else:
                    rc = lib.axon_start_nrt_profile(None, 0)
                if rc != 0:
                    raise RuntimeError(f"axon_start_nrt_profile rc={rc}")
                try:
                    yield
                finally:
                    n = lib.axon_stop_nrt_profile(str(output_dir).encode())
                    if n <= 0:
                        import sys as _s
                        print(f"profile: rc={n} writing {output_dir}",
                              file=_s.stderr)

            hook = _hook
    except OSError:
        pass

    mod = types.ModuleType("antenv.axon_hooks")
    _state = {"hook": hook}
    mod.get_axon_ntff_profile_hook = lambda: _state["hook"]
    mod.set_axon_ntff_profile_hook = lambda h: _state.__setitem__("hook", h)
    sys.modules["antenv.axon_hooks"] = mod
    try:
        import antenv
        antenv.axon_hooks = mod
    except ImportError:
        pass


def _build_nc():
    import concourse.bass as bass  # noqa: F401
    import concourse.tile as tile
    from concourse import bacc, mybir

    f32 = mybir.dt.float32
    nc = bacc.Bacc("TRN2", target_bir_lowering=False, debug=False,
                   num_devices=NCORES)

    x_d = nc.dram_tensor("inputs", [BLOC, M, I], f32, kind="ExternalInput").ap()
    w_d = nc.dram_tensor("w_pci", [128, C, I], f32, kind="ExternalInput").ap()
    b_d = nc.dram_tensor("bias_pc", [128, C], f32, kind="ExternalInput").ap()
    bf_d = nc.dram_tensor("biasf_pc", [128, C], f32, kind="ExternalInput").ap()
    out_d = nc.dram_tensor("out", [BLOC, M], f32, kind="ExternalOutput").ap()
    ident_d = nc.inline_tensor(np.eye(128, dtype=np.float32), name="ident")

    mult = mybir.AluOpType.mult
    add = mybir.AluOpType.add
    ident_fn = mybir.ActivationFunctionType.Identity

    with tile.TileContext(nc) as tc:
        with tc.tile_pool(name="const", bufs=1) as cpool, \
             tc.tile_pool(name="xin", bufs=3) as xpool, \
             tc.tile_pool(name="prodp", bufs=3) as ppool, \
             tc.tile_pool(name="scrp", bufs=4) as spool, \
             tc.tile_pool(name="accp", bufs=1) as apool, \
             tc.tile_pool(name="outp", bufs=1) as opool, \
             tc.tile_pool(name="tpp", bufs=2, space="PSUM") as tppool:

            w_sb = cpool.tile([128, C, I], f32, name="w_sb")
            nc.sync.dma_start(w_sb[:], w_d[:])
            bias_sb = cpool.tile([128, C], f32, name="bias_sb")
            nc.sync.dma_start(bias_sb[:], b_d[:])
            biasf_sb = cpool.tile([128, C], f32, name="biasf_sb")
            nc.sync.dma_start(biasf_sb[:], bf_d[:])
            ident_sb = cpool.tile([128, 128], f32, name="ident_sb")
            nc.sync.dma_start(ident_sb[:], ident_d.ap())

            # results land here as [m_p, b, c]
            acc = apool.tile([128, BLOC, C], f32, name="acc")

            # [b, (c p), i] -> [p, b, c, i]
            xv = x_d.rearrange("b (c p) i -> p b c i", p=128)

            for blk in range(BLOC // NB):
                b0 = blk * NB
                xt = xpool.tile([128, NB, C, I], f32, name="xt", tag="xt")
                nc.sync.dma_start(xt[:], xv[:, b0:b0 + NB])
                for j in range(NB):
                    b = b0 + j
                    if b % 3 == 0:
                        # DVE-only path: mult, segmented reduce, bias add
                        prodd = ppool.tile([128, C, I], f32, name="prodd",
                                           tag="prod")
                        nc.vector.tensor_mul(prodd[:], xt[:, j], w_sb[:])
                        nc.vector.tensor_reduce(
                            out=acc[:, b],
                            in_=prodd[:],
                            axis=mybir.AxisListType.X,
                            op=add,
                        )
                        nc.vector.tensor_add(acc[:, b], acc[:, b], bias_sb[:])
                    else:
                        # DVE multiplies, ScalarE reduces (+bias/128 per elem)
                        prod = ppool.tile([128, C, I], f32, name="prod",
                                          tag="prod")
                        nc.vector.tensor_mul(prod[:], xt[:, j], w_sb[:])
                        for c in range(C):
                            scr2 = spool.tile([128, I], f32, name="scr2",
                                              tag="scr2")
                            nc.scalar.activation(
                                out=scr2[:],
                                in_=prod[:, c],
                                func=ident_fn,
                                bias=biasf_sb[:, c:c + 1],
                                scale=1.0,
                                accum_out=acc[:, b, c:c + 1],
                            )

            # acc [m_p, b, c] -> out_sb [b_p, c, m'] via PE transposes
            out_sb = opool.tile([128, C, 128], f32, name="out_sb")
            for c in range(C):
                tp = tppool.tile([128, 128], f32, name="tp", tag="tp")
                nc.tensor.transpose(tp[:], acc[:, :, c], ident_sb[:])
                nc.vector.tensor_copy(out_sb[:, c], tp[:])
            nc.sync.dma_start(out_d.rearrange("b (c p) -> b c p", p=128),
                              out_sb[:])

    nc.compile()
    return nc


def _get_nc():
    if "nc" not in _CACHE:
        _CACHE["nc"] = _build_nc()
    return _CACHE["nc"]


def kernel(inputs, Rk_weight, bias):
    global LAST_RESULT
    _ensure_ntff_hook()
    from concourse.bass_utils import run_bass_kernel_spmd

    nc = _get_nc()

    inputs = np.ascontiguousarray(inputs, dtype=np.float32)
    w_pci = np.ascontiguousarray(
        Rk_weight.reshape(C, 128, I).transpose(1, 0, 2), dtype=np.float32)
    bias_pc = np.ascontiguousarray(
        bias.reshape(C, 128).T, dtype=np.float32)
    biasf_pc = np.ascontiguousarray(bias_pc / np.float32(I))

    in_maps = []
    for core in range(NCORES):
        in_maps.append({
            "inputs": inputs[core * BLOC:(core + 1) * BLOC],
            "w_pci": w_pci,
            "bias_pc": bias_pc,
            "biasf_pc": biasf_pc,
        })

    res = run_bass_kernel_spmd(nc, in_maps, list(range(NCORES)))
    LAST_RESULT = res
    out = np.concatenate(
        [np.asarray(res.results[i]["out"]) for i in range(NCORES)], axis=0)
    return out.astype(np.float32, copy=False)
